# revision 9
# baseline (speedup 1.0000x reference)
"""Bottom-k cross-entropy loss on 8 Trainium2 NeuronCores.

Per-sample CE over [8192, 32000] logits, then mean of the 4096 smallest
losses.  Data-parallel: rows sharded across 8 cores; each core streams its
131MB shard once (memory-bound), computes local CE via one fused
exp+accumulate pass on the scalar engine, all-gathers the 8192 losses
(tiny), and every core redundantly runs an exact threshold-refinement
selection (3 rounds x 128 brackets, then a tie-corrected min-sum) to
produce the bottom-k mean.

Selection math: brackets are multiples of 2^-16 < 32, so all threshold
arithmetic is exact in f32.  With t >= v_(m) within one final bracket,
  mean_bottom_m = (sum_i min(x_i, t) - (N - m) * t) / m
is exact up to (C(t)-m)*bracket_width/m < 1e-7.
"""

import numpy as np

N_CORES = 8
N_FULL, V_FULL = 8192, 32000
P = 128

# bracket steps: ranges 32, 0.25, 2^-9; all CE values lie in (0, 32]
S1, S2, S3 = 2.0**-2, 2.0**-9, 2.0**-16


def build_nc(n_cores, r, v, f):
    """Build the SPMD Bass program (identical on every core)."""
    from concourse import bass, bacc, mybir, tile

    assert r % P == 0 and v % f == 0
    rb_n = r // P
    nch = v // f
    ng = r * n_cores
    m = ng // 2
    f32 = mybir.dt.float32

    nc = bacc.Bacc()
    x = nc.declare_dram_parameter("x", [r, v], f32, isOutput=False)
    offs = nc.declare_dram_parameter("offs", [P, rb_n], mybir.dt.int32, isOutput=False)
    iota1 = nc.declare_dram_parameter("iota1", [P, 1], f32, isOutput=False)
    iota2 = nc.declare_dram_parameter("iota2", [P, 1], f32, isOutput=False)
    iota3 = nc.declare_dram_parameter("iota3", [P, 1], f32, isOutput=False)
    out = nc.declare_dram_parameter("out", [1, 1], f32, isOutput=True)

    with tile.TileContext(nc) as tc:
        with (
            tc.tile_pool(name="dram", bufs=1, space="DRAM") as dpool,
            tc.tile_pool(name="consts", bufs=1) as cpool,
            tc.tile_pool(name="xs", bufs=3) as xpool,
            tc.tile_pool(name="es", bufs=2) as epool,
            tc.tile_pool(name="stats", bufs=2) as spool,
            tc.tile_pool(name="sel", bufs=1) as selpool,
            tc.tile_pool(name="psum", bufs=2, space="PSUM") as ppool,
        ):
            ce_local = dpool.tile([r, 1], f32, name="ce_local")
            ce_all = dpool.tile([ng, 1], f32, addr_space="Shared", name="ce_all")
            offs_sb = cpool.tile([P, rb_n], mybir.dt.int32)
            nc.sync.dma_start(offs_sb[:], offs[:])
            io1 = cpool.tile([P, 1], f32)
            nc.sync.dma_start(io1[:], iota1[:])
            io2 = cpool.tile([P, 1], f32)
            nc.sync.dma_start(io2[:], iota2[:])
            io3 = cpool.tile([P, 1], f32)
            nc.sync.dma_start(io3[:], iota3[:])

            # gather picked logits: x.flat[row*v + label] for each local row
            picked = cpool.tile([P, rb_n], f32)
            x_flat = x[:].rearrange("a b -> (a b) ()")
            for rbi in range(rb_n):
                nc.gpsimd.indirect_dma_start(
                    out=picked[:, rbi : rbi + 1],
                    out_offset=None,
                    in_=x_flat,
                    in_offset=bass.IndirectOffsetOnAxis(
                        ap=offs_sb[:, rbi : rbi + 1], axis=0
                    ),
                )

            # streaming pass: per row-block, sum(exp(x)) via fused ACT accumulate
            for rbi in range(rb_n):
                part = spool.tile([P, nch], f32, tag="part")
                for c in range(nch):
                    xt = xpool.tile([P, f], f32, tag="xt")
                    nc.sync.dma_start(
                        xt[:], x[rbi * P : (rbi + 1) * P, c * f : (c + 1) * f]
                    )
                    esc = epool.tile([P, f], f32, tag="esc")
                    nc.scalar.activation(
                        out=esc[:],
                        in_=xt[:],
                        func=mybir.ActivationFunctionType.Exp,
                        accum_out=part[:, c : c + 1],
                    )
                s_row = spool.tile([P, 1], f32, tag="s_row")
                nc.vector.reduce_sum(s_row[:], part[:], axis=mybir.AxisListType.X)
                logz = spool.tile([P, 1], f32, tag="logz")
                nc.scalar.activation(
                    out=logz[:], in_=s_row[:], func=mybir.ActivationFunctionType.Ln
                )
                ce = spool.tile([P, 1], f32, tag="ce")
                nc.vector.tensor_tensor(
                    out=ce[:],
                    in0=logz[:],
                    in1=picked[:, rbi : rbi + 1],
                    op=mybir.AluOpType.subtract,
                )
                nc.sync.dma_start(ce_local[rbi * P : (rbi + 1) * P, :], ce[:])

            # all-gather the per-sample losses (tiny)
            nc.gpsimd.collective_compute(
                "AllGather",
                mybir.AluOpType.bypass,
                replica_groups=[list(range(n_cores))],
                ins=[ce_local[:].opt()],
                outs=[ce_all[:].opt()],
            )

            # replicate all ng losses into every partition
            xrep = selpool.tile([P, ng], f32)
            nc.sync.dma_start(
                xrep[:], ce_all[:].rearrange("a 1 -> 1 a").to_broadcast([P, ng])
            )

            dummy = selpool.tile([P, 1], f32)
            ones = selpool.tile([P, P], f32)
            nc.vector.memset(ones[:], 1.0)
            fm = float(m)

            def count_round(t_ap, name):
                cnt = selpool.tile([P, 1], f32, name=f"cnt{name}")
                nc.vector.tensor_scalar(
                    out=dummy[:].broadcast_to([P, ng]),
                    in0=xrep[:],
                    scalar1=t_ap,
                    scalar2=None,
                    op0=mybir.AluOpType.is_le,
                    op1=mybir.AluOpType.add,
                    accum_out=cnt[:],
                )
                ge = selpool.tile([P, 1], f32, name=f"ge{name}")
                nc.vector.tensor_scalar(
                    out=ge[:],
                    in0=cnt[:],
                    scalar1=fm,
                    scalar2=None,
                    op0=mybir.AluOpType.is_ge,
                )
                # partition-sum of ge, replicated to all partitions, via ones-matmul
                g = ppool.tile([P, 1], f32, name=f"g{name}", tag="gps")
                nc.tensor.matmul(out=g[:], lhsT=ones[:], rhs=ge[:], start=True, stop=True)
                return g

            # round 1: thresholds io1 = (p+1)*S1; biased lo1 = -g1*S1
            g1 = count_round(io1[:], "1")
            lo1 = selpool.tile([P, 1], f32)
            nc.vector.tensor_scalar(
                out=lo1[:], in0=g1[:], scalar1=-S1, scalar2=None,
                op0=mybir.AluOpType.mult,
            )
            # round 2: T2 = lo1 + (range1 + (p+1)*S2)
            t2 = selpool.tile([P, 1], f32)
            nc.vector.tensor_tensor(
                out=t2[:], in0=lo1[:], in1=io2[:], op=mybir.AluOpType.add
            )
            g2 = count_round(t2[:], "2")
            lo2 = selpool.tile([P, 1], f32)
            nc.vector.tensor_scalar(
                out=lo2[:], in0=g2[:], scalar1=-S2, scalar2=lo1[:],
                op0=mybir.AluOpType.mult, op1=mybir.AluOpType.add,
            )
            # round 3: T3 = lo2 + (range1 + range2 + (p+1)*S3)
            t3 = selpool.tile([P, 1], f32)
            nc.vector.tensor_tensor(
                out=t3[:], in0=lo2[:], in1=io3[:], op=mybir.AluOpType.add
            )
            g3 = count_round(t3[:], "3")
            lo3 = selpool.tile([P, 1], f32)
            nc.vector.tensor_scalar(
                out=lo3[:], in0=g3[:], scalar1=-S3, scalar2=lo2[:],
                op0=mybir.AluOpType.mult, op1=mybir.AluOpType.add,
            )
            # final threshold t = true_lo3 + S3 (un-bias by the three ranges)
            c_t = 128.0 * S1 + 128.0 * S2 + 128.0 * S3 + S3
            tf = selpool.tile([P, 1], f32)
            nc.vector.tensor_scalar(
                out=tf[:], in0=lo3[:], scalar1=c_t, scalar2=None,
                op0=mybir.AluOpType.add,
            )
            # tie-corrected bottom-m mean: (sum(min(x, t)) - (ng - m)*t) / m
            # chunked accumulation to keep f32 rounding ~sqrt(8) lower
            n_sc = 8
            assert ng % n_sc == 0
            sc = ng // n_sc
            smin_cols = selpool.tile([P, n_sc], f32)
            for k in range(n_sc):
                nc.vector.tensor_scalar(
                    out=dummy[:].broadcast_to([P, sc]),
                    in0=xrep[:, k * sc : (k + 1) * sc],
                    scalar1=tf[:],
                    scalar2=None,
                    op0=mybir.AluOpType.min,
                    op1=mybir.AluOpType.add,
                    accum_out=smin_cols[:, k : k + 1],
                )
            smin = selpool.tile([P, 1], f32)
            nc.vector.reduce_sum(smin[:], smin_cols[:], axis=mybir.AxisListType.X)
            res = selpool.tile([P, 1], f32)
            # res = smin/m - t * (ng - m)/m ; with m = ng/2 this is smin/m - t
            assert ng == 2 * m
            nc.vector.tensor_scalar(
                out=res[:], in0=smin[:], scalar1=1.0 / m, scalar2=tf[:],
                op0=mybir.AluOpType.mult, op1=mybir.AluOpType.subtract,
            )
            nc.sync.dma_start(out[:], res[0:1, :])

    if not nc.is_finalized():
        nc.finalize()
    return nc


def make_host_inputs(x_full, labels_full, n_cores, r, v):
    """Shard rows across cores and build the per-core input maps."""
    rb_n = r // P
    io1 = ((np.arange(P, dtype=np.float64) + 1) * S1).astype(np.float32)
    io2 = (128 * S1 + (np.arange(P, dtype=np.float64) + 1) * S2).astype(np.float32)
    io3 = (128 * S1 + 128 * S2 + (np.arange(P, dtype=np.float64) + 1) * S3).astype(
        np.float32
    )
    in_maps = []
    for c in range(n_cores):
        rows = slice(c * r, (c + 1) * r)
        xs = np.ascontiguousarray(x_full[rows], dtype=np.float32)
        lb = np.asarray(labels_full[rows], dtype=np.int64)
        offs_flat = (np.arange(r, dtype=np.int64) * v + lb).astype(np.int32)
        offs = np.ascontiguousarray(offs_flat.reshape(rb_n, P).T)
        in_maps.append(
            {
                "x": xs,
                "offs": offs,
                "iota1": io1.reshape(P, 1),
                "iota2": io2.reshape(P, 1),
                "iota3": io3.reshape(P, 1),
            }
        )
    return in_maps


def run(inputs, trace=False, f=4000):
    from concourse.bass_utils import run_bass_kernel_spmd

    x_full = np.asarray(inputs["outputs"], dtype=np.float32)
    labels_full = np.asarray(inputs["labels"])
    n, v = x_full.shape
    r = n // N_CORES
    nc = build_nc(N_CORES, r, v, f)
    in_maps = make_host_inputs(x_full, labels_full, N_CORES, r, v)
    res = run_bass_kernel_spmd(
        nc, in_maps, list(range(N_CORES)), trace=trace
    )
    val = np.asarray(res.results[0]["out"], dtype=np.float32).reshape(-1)[0]
    return np.asarray(val, dtype=np.float32), res


def kernel(outputs=None, labels=None, **_ignored):
    out, _ = run({"outputs": outputs, "labels": labels})
    return out


# revision 14
# speedup vs baseline: 1.2786x; 1.2786x over previous
"""Bottom-k cross-entropy loss on 8 Trainium2 NeuronCores.

Per-sample CE over [8192, 32000] logits, then mean of the 4096 smallest
losses.  Data-parallel: rows sharded across 8 cores; each core streams its
131MB shard once (memory-bound), computes local CE via one fused
exp+accumulate pass on the scalar engine, all-gathers the 8192 losses
(tiny), and every core redundantly runs an exact threshold-refinement
selection (3 rounds x 128 brackets, then a tie-corrected min-sum) to
produce the bottom-k mean.

Selection math: brackets are multiples of 2^-16 < 32, so all threshold
arithmetic is exact in f32.  With t >= v_(m) within one final bracket,
  mean_bottom_m = (sum_i min(x_i, t) - (N - m) * t) / m
is exact up to (C(t)-m)*bracket_width/m < 1e-7.
"""

import numpy as np

N_CORES = 8
N_FULL, V_FULL = 8192, 32000
P = 128

# bracket steps: ranges 32, 0.25, 2^-9; all CE values lie in (0, 32]
S1, S2, S3 = 2.0**-2, 2.0**-9, 2.0**-16


def build_nc(n_cores, r, v, f):
    """Build the SPMD Bass program (identical on every core)."""
    from concourse import bass, bacc, mybir, tile

    assert r % P == 0 and v % f == 0
    rb_n = r // P
    nch = v // f
    ng = r * n_cores
    m = ng // 2
    f32 = mybir.dt.float32

    nc = bacc.Bacc()
    x = nc.declare_dram_parameter("x", [r, v], f32, isOutput=False)
    offs = nc.declare_dram_parameter("offs", [P, rb_n], mybir.dt.int32, isOutput=False)
    iota1 = nc.declare_dram_parameter("iota1", [P, 1], f32, isOutput=False)
    iota2 = nc.declare_dram_parameter("iota2", [P, 1], f32, isOutput=False)
    iota3 = nc.declare_dram_parameter("iota3", [P, 1], f32, isOutput=False)
    out = nc.declare_dram_parameter("out", [1, 1], f32, isOutput=True)

    with tile.TileContext(nc) as tc:
        with (
            tc.tile_pool(name="dram", bufs=1, space="DRAM") as dpool,
            tc.tile_pool(name="consts", bufs=1) as cpool,
            tc.tile_pool(name="xs", bufs=4) as xpool,
            tc.tile_pool(name="es", bufs=2) as epool,
            tc.tile_pool(name="stats", bufs=2) as spool,
            tc.tile_pool(name="sel", bufs=1) as selpool,
            tc.tile_pool(name="psum", bufs=2, space="PSUM") as ppool,
        ):
            ce_local = dpool.tile([r, 1], f32, name="ce_local")
            ce_all = dpool.tile([ng, 1], f32, addr_space="Shared", name="ce_all")
            offs_sb = cpool.tile([P, rb_n], mybir.dt.int32)
            nc.gpsimd.dma_start(offs_sb[:], offs[:])
            io1 = cpool.tile([P, 1], f32)
            nc.gpsimd.dma_start(io1[:], iota1[:])
            io2 = cpool.tile([P, 1], f32)
            nc.gpsimd.dma_start(io2[:], iota2[:])
            io3 = cpool.tile([P, 1], f32)
            nc.gpsimd.dma_start(io3[:], iota3[:])

            # gather picked logits: x.flat[row*v + label] for each local row
            picked = cpool.tile([P, rb_n], f32)
            x_flat = x[:].rearrange("a b -> (a b) ()")
            for rbi in range(rb_n):
                nc.gpsimd.indirect_dma_start(
                    out=picked[:, rbi : rbi + 1],
                    out_offset=None,
                    in_=x_flat,
                    in_offset=bass.IndirectOffsetOnAxis(
                        ap=offs_sb[:, rbi : rbi + 1], axis=0
                    ),
                )

            # streaming pass: pure DMA + fused exp/accumulate, no mid-stream
            # epilogues (table switches and dependent stores would stall the
            # sync HWDGE ring and the ACT queue at row-block boundaries)
            part_all = spool.tile([P, rb_n * nch], f32)
            for rbi in range(rb_n):
                for c in range(nch):
                    xt = xpool.tile([P, f], f32, tag="xt")
                    nc.sync.dma_start(
                        xt[:], x[rbi * P : (rbi + 1) * P, c * f : (c + 1) * f]
                    )
                    esc = epool.tile([P, f], f32, tag="esc")
                    nc.scalar.activation(
                        out=esc[:],
                        in_=xt[:],
                        func=mybir.ActivationFunctionType.Exp,
                        accum_out=part_all[:, rbi * nch + c : rbi * nch + c + 1],
                    )
            # batched epilogue for all row blocks at once
            s_all = spool.tile([P, rb_n], f32)
            nc.vector.tensor_reduce(
                s_all[:],
                part_all[:].rearrange("p (b c) -> p b c", c=nch),
                axis=mybir.AxisListType.X,
                op=mybir.AluOpType.add,
            )
            logz_all = spool.tile([P, rb_n], f32)
            nc.scalar.activation(
                out=logz_all[:], in_=s_all[:], func=mybir.ActivationFunctionType.Ln
            )
            ce_sb = spool.tile([P, rb_n], f32)
            nc.vector.tensor_tensor(
                out=ce_sb[:],
                in0=logz_all[:],
                in1=picked[:],
                op=mybir.AluOpType.subtract,
            )
            # p-major layout into ce_local (any permutation is fine: the
            # bottom-k mean is permutation invariant)
            nc.sync.dma_start(
                ce_local[:].rearrange("(p b) 1 -> p b", b=rb_n), ce_sb[:]
            )

            # all-gather the per-sample losses (tiny)
            nc.gpsimd.collective_compute(
                "AllGather",
                mybir.AluOpType.bypass,
                replica_groups=[list(range(n_cores))],
                ins=[ce_local[:].opt()],
                outs=[ce_all[:].opt()],
            )

            # replicate all ng losses into every partition (DRAM source reread
            # per partition; partition-step-0 SBUF sources are not allowed)
            xrep = selpool.tile([P, ng], f32)
            nc.sync.dma_start(
                xrep[:], ce_all[:].rearrange("a 1 -> 1 a").to_broadcast([P, ng])
            )

            dummy = selpool.tile([P, 1], f32)
            ones = selpool.tile([P, P], f32)
            nc.vector.memset(ones[:], 1.0)
            fm = float(m)

            def count_round(t_ap, name):
                cnt = selpool.tile([P, 1], f32, name=f"cnt{name}")
                nc.vector.tensor_scalar(
                    out=dummy[:].broadcast_to([P, ng]),
                    in0=xrep[:],
                    scalar1=t_ap,
                    scalar2=None,
                    op0=mybir.AluOpType.is_le,
                    op1=mybir.AluOpType.add,
                    accum_out=cnt[:],
                )
                ge = selpool.tile([P, 1], f32, name=f"ge{name}")
                nc.vector.tensor_scalar(
                    out=ge[:],
                    in0=cnt[:],
                    scalar1=fm,
                    scalar2=None,
                    op0=mybir.AluOpType.is_ge,
                )
                # partition-sum of ge, replicated to all partitions, via ones-matmul
                g = ppool.tile([P, 1], f32, name=f"g{name}", tag="gps")
                nc.tensor.matmul(out=g[:], lhsT=ones[:], rhs=ge[:], start=True, stop=True)
                return g

            # round 1: thresholds io1 = (p+1)*S1; biased lo1 = -g1*S1
            g1 = count_round(io1[:], "1")
            lo1 = selpool.tile([P, 1], f32)
            nc.vector.tensor_scalar(
                out=lo1[:], in0=g1[:], scalar1=-S1, scalar2=None,
                op0=mybir.AluOpType.mult,
            )
            # round 2: T2 = lo1 + (range1 + (p+1)*S2)
            t2 = selpool.tile([P, 1], f32)
            nc.vector.tensor_tensor(
                out=t2[:], in0=lo1[:], in1=io2[:], op=mybir.AluOpType.add
            )
            g2 = count_round(t2[:], "2")
            lo2 = selpool.tile([P, 1], f32)
            nc.vector.tensor_scalar(
                out=lo2[:], in0=g2[:], scalar1=-S2, scalar2=lo1[:],
                op0=mybir.AluOpType.mult, op1=mybir.AluOpType.add,
            )
            # round 3: T3 = lo2 + (range1 + range2 + (p+1)*S3)
            t3 = selpool.tile([P, 1], f32)
            nc.vector.tensor_tensor(
                out=t3[:], in0=lo2[:], in1=io3[:], op=mybir.AluOpType.add
            )
            g3 = count_round(t3[:], "3")
            lo3 = selpool.tile([P, 1], f32)
            nc.vector.tensor_scalar(
                out=lo3[:], in0=g3[:], scalar1=-S3, scalar2=lo2[:],
                op0=mybir.AluOpType.mult, op1=mybir.AluOpType.add,
            )
            # final threshold t = true_lo3 + S3 (un-bias by the three ranges)
            c_t = 128.0 * S1 + 128.0 * S2 + 128.0 * S3 + S3
            tf = selpool.tile([P, 1], f32)
            nc.vector.tensor_scalar(
                out=tf[:], in0=lo3[:], scalar1=c_t, scalar2=None,
                op0=mybir.AluOpType.add,
            )
            # tie-corrected bottom-m mean: (sum(min(x, t)) - (ng - m)*t) / m
            # chunked accumulation to keep f32 rounding ~sqrt(8) lower
            n_sc = 8
            assert ng % n_sc == 0
            sc = ng // n_sc
            smin_cols = selpool.tile([P, n_sc], f32)
            for k in range(n_sc):
                nc.vector.tensor_scalar(
                    out=dummy[:].broadcast_to([P, sc]),
                    in0=xrep[:, k * sc : (k + 1) * sc],
                    scalar1=tf[:],
                    scalar2=None,
                    op0=mybir.AluOpType.min,
                    op1=mybir.AluOpType.add,
                    accum_out=smin_cols[:, k : k + 1],
                )
            smin = selpool.tile([P, 1], f32)
            nc.vector.reduce_sum(smin[:], smin_cols[:], axis=mybir.AxisListType.X)
            res = selpool.tile([P, 1], f32)
            # res = smin/m - t * (ng - m)/m ; with m = ng/2 this is smin/m - t
            assert ng == 2 * m
            nc.vector.tensor_scalar(
                out=res[:], in0=smin[:], scalar1=1.0 / m, scalar2=tf[:],
                op0=mybir.AluOpType.mult, op1=mybir.AluOpType.subtract,
            )
            nc.sync.dma_start(out[:], res[0:1, :])

    if not nc.is_finalized():
        nc.finalize()
    return nc


def make_host_inputs(x_full, labels_full, n_cores, r, v):
    """Shard rows across cores and build the per-core input maps."""
    rb_n = r // P
    io1 = ((np.arange(P, dtype=np.float64) + 1) * S1).astype(np.float32)
    io2 = (128 * S1 + (np.arange(P, dtype=np.float64) + 1) * S2).astype(np.float32)
    io3 = (128 * S1 + 128 * S2 + (np.arange(P, dtype=np.float64) + 1) * S3).astype(
        np.float32
    )
    in_maps = []
    for c in range(n_cores):
        rows = slice(c * r, (c + 1) * r)
        xs = np.ascontiguousarray(x_full[rows], dtype=np.float32)
        lb = np.asarray(labels_full[rows], dtype=np.int64)
        offs_flat = (np.arange(r, dtype=np.int64) * v + lb).astype(np.int32)
        offs = np.ascontiguousarray(offs_flat.reshape(rb_n, P).T)
        in_maps.append(
            {
                "x": xs,
                "offs": offs,
                "iota1": io1.reshape(P, 1),
                "iota2": io2.reshape(P, 1),
                "iota3": io3.reshape(P, 1),
            }
        )
    return in_maps


def run(inputs, trace=False, f=4000):
    from concourse.bass_utils import run_bass_kernel_spmd

    x_full = np.asarray(inputs["outputs"], dtype=np.float32)
    labels_full = np.asarray(inputs["labels"])
    n, v = x_full.shape
    r = n // N_CORES
    nc = build_nc(N_CORES, r, v, f)
    in_maps = make_host_inputs(x_full, labels_full, N_CORES, r, v)
    res = run_bass_kernel_spmd(
        nc, in_maps, list(range(N_CORES)), trace=trace
    )
    val = np.asarray(res.results[0]["out"], dtype=np.float32).reshape(-1)[0]
    return np.asarray(val, dtype=np.float32), res


def kernel(outputs=None, labels=None, **_ignored):
    out, _ = run({"outputs": outputs, "labels": labels})
    return out


# revision 16
# speedup vs baseline: 1.2852x; 1.0052x over previous
"""Bottom-k cross-entropy loss on 8 Trainium2 NeuronCores.

Per-sample CE over [8192, 32000] logits, then mean of the 4096 smallest
losses.  Data-parallel: rows sharded across 8 cores; each core streams its
131MB shard once (memory-bound), computes local CE via one fused
exp+accumulate pass on the scalar engine, all-gathers the 8192 losses
(tiny), and every core redundantly runs an exact threshold-refinement
selection (3 rounds x 128 brackets, then a tie-corrected min-sum) to
produce the bottom-k mean.

Selection math: brackets are multiples of 2^-16 < 32, so all threshold
arithmetic is exact in f32.  With t >= v_(m) within one final bracket,
  mean_bottom_m = (sum_i min(x_i, t) - (N - m) * t) / m
is exact up to (C(t)-m)*bracket_width/m < 1e-7.
"""

import numpy as np

N_CORES = 8
N_FULL, V_FULL = 8192, 32000
P = 128

# bracket steps: ranges 32, 0.25, 2^-9; all CE values lie in (0, 32]
S1, S2, S3 = 2.0**-2, 2.0**-9, 2.0**-16


def build_nc(n_cores, r, v, f):
    """Build the SPMD Bass program (identical on every core)."""
    from concourse import bass, bacc, mybir, tile

    assert r % P == 0 and v % f == 0
    rb_n = r // P
    nch = v // f
    ng = r * n_cores
    m = ng // 2
    f32 = mybir.dt.float32

    nc = bacc.Bacc()
    x = nc.declare_dram_parameter("x", [r, v], f32, isOutput=False)
    offs = nc.declare_dram_parameter("offs", [P, rb_n], mybir.dt.int32, isOutput=False)
    iota1 = nc.declare_dram_parameter("iota1", [P, 1], f32, isOutput=False)
    iota2 = nc.declare_dram_parameter("iota2", [P, 1], f32, isOutput=False)
    iota3 = nc.declare_dram_parameter("iota3", [P, 1], f32, isOutput=False)
    out = nc.declare_dram_parameter("out", [1, 1], f32, isOutput=True)

    with tile.TileContext(nc) as tc:
        with (
            tc.tile_pool(name="dram", bufs=1, space="DRAM") as dpool,
            tc.tile_pool(name="consts", bufs=1) as cpool,
            tc.tile_pool(name="xs", bufs=4) as xpool,
            tc.tile_pool(name="es", bufs=2) as epool,
            tc.tile_pool(name="stats", bufs=2) as spool,
            tc.tile_pool(name="sel", bufs=1) as selpool,
            tc.tile_pool(name="psum", bufs=2, space="PSUM") as ppool,
        ):
            ce_local = dpool.tile([r, 1], f32, name="ce_local")
            ce_all = dpool.tile([ng, 1], f32, addr_space="Shared", name="ce_all")
            offs_sb = cpool.tile([P, rb_n], mybir.dt.int32)
            nc.gpsimd.dma_start(offs_sb[:], offs[:])
            io1 = cpool.tile([P, 1], f32)
            nc.gpsimd.dma_start(io1[:], iota1[:])
            io2 = cpool.tile([P, 1], f32)
            nc.gpsimd.dma_start(io2[:], iota2[:])
            io3 = cpool.tile([P, 1], f32)
            nc.gpsimd.dma_start(io3[:], iota3[:])

            # gather picked logits: x.flat[row*v + label] for each local row
            picked = cpool.tile([P, rb_n], f32)
            x_flat = x[:].rearrange("a b -> (a b) ()")
            for rbi in range(rb_n):
                nc.gpsimd.indirect_dma_start(
                    out=picked[:, rbi : rbi + 1],
                    out_offset=None,
                    in_=x_flat,
                    in_offset=bass.IndirectOffsetOnAxis(
                        ap=offs_sb[:, rbi : rbi + 1], axis=0
                    ),
                )

            # streaming pass: pure DMA + fused exp/accumulate, no mid-stream
            # epilogues (table switches and dependent stores would stall the
            # sync HWDGE ring and the ACT queue at row-block boundaries)
            part_all = spool.tile([P, rb_n * nch], f32)
            for rbi in range(rb_n):
                for c in range(nch):
                    xt = xpool.tile([P, f], f32, tag="xt")
                    nc.sync.dma_start(
                        xt[:], x[rbi * P : (rbi + 1) * P, c * f : (c + 1) * f]
                    )
                    esc = epool.tile([P, f], f32, tag="esc")
                    nc.scalar.activation(
                        out=esc[:],
                        in_=xt[:],
                        func=mybir.ActivationFunctionType.Exp,
                        accum_out=part_all[:, rbi * nch + c : rbi * nch + c + 1],
                    )
            # batched epilogue for all row blocks at once
            s_all = spool.tile([P, rb_n], f32)
            nc.vector.tensor_reduce(
                s_all[:],
                part_all[:].rearrange("p (b c) -> p b c", c=nch),
                axis=mybir.AxisListType.X,
                op=mybir.AluOpType.add,
            )
            logz_all = spool.tile([P, rb_n], f32)
            nc.scalar.activation(
                out=logz_all[:], in_=s_all[:], func=mybir.ActivationFunctionType.Ln
            )
            ce_sb = spool.tile([P, rb_n], f32)
            nc.vector.tensor_tensor(
                out=ce_sb[:],
                in0=logz_all[:],
                in1=picked[:],
                op=mybir.AluOpType.subtract,
            )
            # p-major layout into ce_local (any permutation is fine: the
            # bottom-k mean is permutation invariant)
            nc.sync.dma_start(
                ce_local[:].rearrange("(p b) 1 -> p b", b=rb_n), ce_sb[:]
            )

            # all-gather the per-sample losses (tiny)
            nc.gpsimd.collective_compute(
                "AllGather",
                mybir.AluOpType.bypass,
                replica_groups=[list(range(n_cores))],
                ins=[ce_local[:].opt()],
                outs=[ce_all[:].opt()],
            )

            # replicate all ng losses into every partition via K=1 ones-matmul
            # (weights are exactly 1.0 in bf16, so values pass through exactly;
            # much faster than a partition-broadcast DMA rereading DRAM 128x)
            ce_row = selpool.tile([1, ng], f32)
            nc.sync.dma_start(ce_row[:], ce_all[:].rearrange("a 1 -> 1 a"))
            ones_row = selpool.tile([1, P], f32)
            nc.vector.memset(ones_row[:], 1.0)
            xrep = selpool.tile([P, ng], f32)
            n_bc = (ng + 511) // 512
            for j in range(n_bc):
                js = slice(j * 512, min((j + 1) * 512, ng))
                w = js.stop - js.start
                psb = ppool.tile([P, 512], f32, tag="bc", name=f"bc{j}")
                nc.tensor.matmul(
                    out=psb[:, :w], lhsT=ones_row[:], rhs=ce_row[:, js],
                    start=True, stop=True,
                )
                if j % 2 == 0:
                    nc.vector.tensor_copy(out=xrep[:, js], in_=psb[:, :w])
                else:
                    nc.scalar.copy(out=xrep[:, js], in_=psb[:, :w])

            dummy = selpool.tile([P, 1], f32)
            ones = selpool.tile([P, P], f32)
            nc.vector.memset(ones[:], 1.0)
            fm = float(m)

            def count_round(t_ap, name):
                cnt = selpool.tile([P, 1], f32, name=f"cnt{name}")
                nc.vector.tensor_scalar(
                    out=dummy[:].broadcast_to([P, ng]),
                    in0=xrep[:],
                    scalar1=t_ap,
                    scalar2=None,
                    op0=mybir.AluOpType.is_le,
                    op1=mybir.AluOpType.add,
                    accum_out=cnt[:],
                )
                ge = selpool.tile([P, 1], f32, name=f"ge{name}")
                nc.vector.tensor_scalar(
                    out=ge[:],
                    in0=cnt[:],
                    scalar1=fm,
                    scalar2=None,
                    op0=mybir.AluOpType.is_ge,
                )
                # partition-sum of ge, replicated to all partitions, via ones-matmul
                g = ppool.tile([P, 1], f32, name=f"g{name}", tag="gps")
                nc.tensor.matmul(out=g[:], lhsT=ones[:], rhs=ge[:], start=True, stop=True)
                return g

            # round 1: thresholds io1 = (p+1)*S1; biased lo1 = -g1*S1
            g1 = count_round(io1[:], "1")
            lo1 = selpool.tile([P, 1], f32)
            nc.vector.tensor_scalar(
                out=lo1[:], in0=g1[:], scalar1=-S1, scalar2=None,
                op0=mybir.AluOpType.mult,
            )
            # round 2: T2 = lo1 + (range1 + (p+1)*S2)
            t2 = selpool.tile([P, 1], f32)
            nc.vector.tensor_tensor(
                out=t2[:], in0=lo1[:], in1=io2[:], op=mybir.AluOpType.add
            )
            g2 = count_round(t2[:], "2")
            lo2 = selpool.tile([P, 1], f32)
            nc.vector.tensor_scalar(
                out=lo2[:], in0=g2[:], scalar1=-S2, scalar2=lo1[:],
                op0=mybir.AluOpType.mult, op1=mybir.AluOpType.add,
            )
            # final threshold t = true_lo2 + S2 (un-bias by the two ranges).
            # Bracket width S2 ~ 2e-3: the min-sum correction keeps the
            # result error ~ (#ties within S2 of v_m) * S2 / m ~ 1e-6 abs.
            c_t = 128.0 * S1 + 128.0 * S2 + S2
            tf = selpool.tile([P, 1], f32)
            nc.vector.tensor_scalar(
                out=tf[:], in0=lo2[:], scalar1=c_t, scalar2=None,
                op0=mybir.AluOpType.add,
            )
            # tie-corrected bottom-m mean: (sum(min(x, t)) - (ng - m)*t) / m
            # chunked accumulation to keep f32 rounding ~sqrt(8) lower
            n_sc = 8
            assert ng % n_sc == 0
            sc = ng // n_sc
            smin_cols = selpool.tile([P, n_sc], f32)
            for k in range(n_sc):
                nc.vector.tensor_scalar(
                    out=dummy[:].broadcast_to([P, sc]),
                    in0=xrep[:, k * sc : (k + 1) * sc],
                    scalar1=tf[:],
                    scalar2=None,
                    op0=mybir.AluOpType.min,
                    op1=mybir.AluOpType.add,
                    accum_out=smin_cols[:, k : k + 1],
                )
            smin = selpool.tile([P, 1], f32)
            nc.vector.reduce_sum(smin[:], smin_cols[:], axis=mybir.AxisListType.X)
            res = selpool.tile([P, 1], f32)
            # res = smin/m - t * (ng - m)/m ; with m = ng/2 this is smin/m - t
            assert ng == 2 * m
            nc.vector.tensor_scalar(
                out=res[:], in0=smin[:], scalar1=1.0 / m, scalar2=tf[:],
                op0=mybir.AluOpType.mult, op1=mybir.AluOpType.subtract,
            )
            nc.sync.dma_start(out[:], res[0:1, :])

    if not nc.is_finalized():
        nc.finalize()
    return nc


def make_host_inputs(x_full, labels_full, n_cores, r, v):
    """Shard rows across cores and build the per-core input maps."""
    rb_n = r // P
    io1 = ((np.arange(P, dtype=np.float64) + 1) * S1).astype(np.float32)
    io2 = (128 * S1 + (np.arange(P, dtype=np.float64) + 1) * S2).astype(np.float32)
    io3 = (128 * S1 + 128 * S2 + (np.arange(P, dtype=np.float64) + 1) * S3).astype(
        np.float32
    )
    in_maps = []
    for c in range(n_cores):
        rows = slice(c * r, (c + 1) * r)
        xs = np.ascontiguousarray(x_full[rows], dtype=np.float32)
        lb = np.asarray(labels_full[rows], dtype=np.int64)
        offs_flat = (np.arange(r, dtype=np.int64) * v + lb).astype(np.int32)
        offs = np.ascontiguousarray(offs_flat.reshape(rb_n, P).T)
        in_maps.append(
            {
                "x": xs,
                "offs": offs,
                "iota1": io1.reshape(P, 1),
                "iota2": io2.reshape(P, 1),
                "iota3": io3.reshape(P, 1),
            }
        )
    return in_maps


def run(inputs, trace=False, f=4000):
    from concourse.bass_utils import run_bass_kernel_spmd

    x_full = np.asarray(inputs["outputs"], dtype=np.float32)
    labels_full = np.asarray(inputs["labels"])
    n, v = x_full.shape
    r = n // N_CORES
    nc = build_nc(N_CORES, r, v, f)
    in_maps = make_host_inputs(x_full, labels_full, N_CORES, r, v)
    res = run_bass_kernel_spmd(
        nc, in_maps, list(range(N_CORES)), trace=trace
    )
    val = np.asarray(res.results[0]["out"], dtype=np.float32).reshape(-1)[0]
    return np.asarray(val, dtype=np.float32), res


def kernel(outputs=None, labels=None, **_ignored):
    out, _ = run({"outputs": outputs, "labels": labels})
    return out


# revision 18
# speedup vs baseline: 1.3423x; 1.0444x over previous
"""Bottom-k cross-entropy loss on 8 Trainium2 NeuronCores.

Per-sample CE over [8192, 32000] logits, then mean of the 4096 smallest
losses.  Data-parallel: rows sharded across 8 cores; each core streams its
131MB shard once (memory-bound), computes local CE via one fused
exp+accumulate pass on the scalar engine, all-gathers the 8192 losses
(tiny), and every core redundantly runs an exact threshold-refinement
selection (3 rounds x 128 brackets, then a tie-corrected min-sum) to
produce the bottom-k mean.

Selection math: brackets are multiples of 2^-16 < 32, so all threshold
arithmetic is exact in f32.  With t >= v_(m) within one final bracket,
  mean_bottom_m = (sum_i min(x_i, t) - (N - m) * t) / m
is exact up to (C(t)-m)*bracket_width/m < 1e-7.
"""

import numpy as np

N_CORES = 8
N_FULL, V_FULL = 8192, 32000
P = 128

# bracket steps: ranges 32, 0.25, 2^-9; all CE values lie in (0, 32]
S1, S2, S3 = 2.0**-2, 2.0**-9, 2.0**-16


def build_nc(n_cores, r, v, f):
    """Build the SPMD Bass program (identical on every core)."""
    from concourse import bass, bacc, mybir, tile

    assert r % P == 0 and v % f == 0
    rb_n = r // P
    nch = v // f
    ng = r * n_cores
    m = ng // 2
    f32 = mybir.dt.float32

    nc = bacc.Bacc()
    x = nc.declare_dram_parameter("x", [r, v], f32, isOutput=False)
    offs = nc.declare_dram_parameter("offs", [P, rb_n], mybir.dt.int32, isOutput=False)
    iota1 = nc.declare_dram_parameter("iota1", [P, 1], f32, isOutput=False)
    iota2 = nc.declare_dram_parameter("iota2", [P, 1], f32, isOutput=False)
    iota3 = nc.declare_dram_parameter("iota3", [P, 1], f32, isOutput=False)
    out = nc.declare_dram_parameter("out", [1, 1], f32, isOutput=True)

    with tile.TileContext(nc) as tc:
        with (
            tc.tile_pool(name="dram", bufs=1, space="DRAM") as dpool,
            tc.tile_pool(name="consts", bufs=1) as cpool,
            tc.tile_pool(name="xs", bufs=4) as xpool,
            tc.tile_pool(name="es", bufs=2) as epool,
            tc.tile_pool(name="stats", bufs=2) as spool,
            tc.tile_pool(name="sel", bufs=1) as selpool,
            tc.tile_pool(name="psum", bufs=2, space="PSUM") as ppool,
        ):
            ce_local = dpool.tile([r, 1], f32, name="ce_local")
            ce_all = dpool.tile([ng, 1], f32, addr_space="Shared", name="ce_all")
            offs_sb = cpool.tile([P, rb_n], mybir.dt.int32)
            nc.gpsimd.dma_start(offs_sb[:], offs[:])
            io1 = cpool.tile([P, 1], f32)
            nc.gpsimd.dma_start(io1[:], iota1[:])
            io2 = cpool.tile([P, 1], f32)
            nc.gpsimd.dma_start(io2[:], iota2[:])
            io3 = cpool.tile([P, 1], f32)
            nc.gpsimd.dma_start(io3[:], iota3[:])

            # tiny dummy partition_broadcast: forces the gpsimd ucode library
            # load to happen here (gpsimd is idle during streaming) instead of
            # right before the real broadcast in the latency-critical tail
            dsrc = cpool.tile([1, 4], f32)
            nc.vector.memset(dsrc[:], 0.0)
            dout = cpool.tile([P, 4], f32)
            nc.gpsimd.partition_broadcast(dout[:], dsrc[:])

            # gather picked logits: x.flat[row*v + label] for each local row
            picked = cpool.tile([P, rb_n], f32)
            x_flat = x[:].rearrange("a b -> (a b) ()")
            for rbi in range(rb_n):
                nc.gpsimd.indirect_dma_start(
                    out=picked[:, rbi : rbi + 1],
                    out_offset=None,
                    in_=x_flat,
                    in_offset=bass.IndirectOffsetOnAxis(
                        ap=offs_sb[:, rbi : rbi + 1], axis=0
                    ),
                )

            # streaming pass: pure DMA + fused exp/accumulate, no mid-stream
            # epilogues (table switches and dependent stores would stall the
            # sync HWDGE ring and the ACT queue at row-block boundaries)
            part_all = spool.tile([P, rb_n * nch], f32)
            for rbi in range(rb_n):
                for c in range(nch):
                    xt = xpool.tile([P, f], f32, tag="xt")
                    nc.sync.dma_start(
                        xt[:], x[rbi * P : (rbi + 1) * P, c * f : (c + 1) * f]
                    )
                    esc = epool.tile([P, f], f32, tag="esc")
                    nc.scalar.activation(
                        out=esc[:],
                        in_=xt[:],
                        func=mybir.ActivationFunctionType.Exp,
                        accum_out=part_all[:, rbi * nch + c : rbi * nch + c + 1],
                    )
            # batched epilogue for all row blocks at once
            s_all = spool.tile([P, rb_n], f32)
            nc.vector.tensor_reduce(
                s_all[:],
                part_all[:].rearrange("p (b c) -> p b c", c=nch),
                axis=mybir.AxisListType.X,
                op=mybir.AluOpType.add,
            )
            logz_all = spool.tile([P, rb_n], f32)
            nc.scalar.activation(
                out=logz_all[:], in_=s_all[:], func=mybir.ActivationFunctionType.Ln
            )
            ce_sb = spool.tile([P, rb_n], f32)
            nc.vector.tensor_tensor(
                out=ce_sb[:],
                in0=logz_all[:],
                in1=picked[:],
                op=mybir.AluOpType.subtract,
            )
            # p-major layout into ce_local (any permutation is fine: the
            # bottom-k mean is permutation invariant)
            nc.sync.dma_start(
                ce_local[:].rearrange("(p b) 1 -> p b", b=rb_n), ce_sb[:]
            )

            # all-gather the per-sample losses (tiny)
            nc.gpsimd.collective_compute(
                "AllGather",
                mybir.AluOpType.bypass,
                replica_groups=[list(range(n_cores))],
                ins=[ce_local[:].opt()],
                outs=[ce_all[:].opt()],
            )

            # replicate all ng losses into every partition (gpsimd ucode
            # cross-lane broadcast of partition 0)
            ce_row = selpool.tile([1, ng], f32)
            nc.sync.dma_start(ce_row[:], ce_all[:].rearrange("a 1 -> 1 a"))
            xrep = selpool.tile([P, ng], f32)
            nc.gpsimd.partition_broadcast(xrep[:], ce_row[:])

            dummy = selpool.tile([P, 1], f32)
            ones = selpool.tile([P, P], f32)
            nc.vector.memset(ones[:], 1.0)
            fm = float(m)

            def count_round(t_ap, name):
                cnt = selpool.tile([P, 1], f32, name=f"cnt{name}")
                nc.vector.tensor_scalar(
                    out=dummy[:].broadcast_to([P, ng]),
                    in0=xrep[:],
                    scalar1=t_ap,
                    scalar2=None,
                    op0=mybir.AluOpType.is_le,
                    op1=mybir.AluOpType.add,
                    accum_out=cnt[:],
                )
                ge = selpool.tile([P, 1], f32, name=f"ge{name}")
                nc.vector.tensor_scalar(
                    out=ge[:],
                    in0=cnt[:],
                    scalar1=fm,
                    scalar2=None,
                    op0=mybir.AluOpType.is_ge,
                )
                # partition-sum of ge, replicated to all partitions, via ones-matmul
                g = ppool.tile([P, 1], f32, name=f"g{name}", tag="gps")
                nc.tensor.matmul(out=g[:], lhsT=ones[:], rhs=ge[:], start=True, stop=True)
                return g

            # round 1: thresholds io1 = (p+1)*S1; biased lo1 = -g1*S1
            g1 = count_round(io1[:], "1")
            lo1 = selpool.tile([P, 1], f32)
            nc.vector.tensor_scalar(
                out=lo1[:], in0=g1[:], scalar1=-S1, scalar2=None,
                op0=mybir.AluOpType.mult,
            )
            # round 2: T2 = lo1 + (range1 + (p+1)*S2)
            t2 = selpool.tile([P, 1], f32)
            nc.vector.tensor_tensor(
                out=t2[:], in0=lo1[:], in1=io2[:], op=mybir.AluOpType.add
            )
            g2 = count_round(t2[:], "2")
            lo2 = selpool.tile([P, 1], f32)
            nc.vector.tensor_scalar(
                out=lo2[:], in0=g2[:], scalar1=-S2, scalar2=lo1[:],
                op0=mybir.AluOpType.mult, op1=mybir.AluOpType.add,
            )
            # final threshold t = true_lo2 + S2 (un-bias by the two ranges).
            # Bracket width S2 ~ 2e-3: the min-sum correction keeps the
            # result error ~ (#ties within S2 of v_m) * S2 / m ~ 1e-6 abs.
            c_t = 128.0 * S1 + 128.0 * S2 + S2
            tf = selpool.tile([P, 1], f32)
            nc.vector.tensor_scalar(
                out=tf[:], in0=lo2[:], scalar1=c_t, scalar2=None,
                op0=mybir.AluOpType.add,
            )
            # tie-corrected bottom-m mean: (sum(min(x, t)) - (ng - m)*t) / m
            # chunked accumulation to keep f32 rounding ~sqrt(8) lower
            n_sc = 8
            assert ng % n_sc == 0
            sc = ng // n_sc
            smin_cols = selpool.tile([P, n_sc], f32)
            for k in range(n_sc):
                nc.vector.tensor_scalar(
                    out=dummy[:].broadcast_to([P, sc]),
                    in0=xrep[:, k * sc : (k + 1) * sc],
                    scalar1=tf[:],
                    scalar2=None,
                    op0=mybir.AluOpType.min,
                    op1=mybir.AluOpType.add,
                    accum_out=smin_cols[:, k : k + 1],
                )
            smin = selpool.tile([P, 1], f32)
            nc.vector.reduce_sum(smin[:], smin_cols[:], axis=mybir.AxisListType.X)
            res = selpool.tile([P, 1], f32)
            # res = smin/m - t * (ng - m)/m ; with m = ng/2 this is smin/m - t
            assert ng == 2 * m
            nc.vector.tensor_scalar(
                out=res[:], in0=smin[:], scalar1=1.0 / m, scalar2=tf[:],
                op0=mybir.AluOpType.mult, op1=mybir.AluOpType.subtract,
            )
            nc.sync.dma_start(out[:], res[0:1, :])

    if not nc.is_finalized():
        nc.finalize()
    return nc


def make_host_inputs(x_full, labels_full, n_cores, r, v):
    """Shard rows across cores and build the per-core input maps."""
    rb_n = r // P
    io1 = ((np.arange(P, dtype=np.float64) + 1) * S1).astype(np.float32)
    io2 = (128 * S1 + (np.arange(P, dtype=np.float64) + 1) * S2).astype(np.float32)
    io3 = (128 * S1 + 128 * S2 + (np.arange(P, dtype=np.float64) + 1) * S3).astype(
        np.float32
    )
    in_maps = []
    for c in range(n_cores):
        rows = slice(c * r, (c + 1) * r)
        xs = np.ascontiguousarray(x_full[rows], dtype=np.float32)
        lb = np.asarray(labels_full[rows], dtype=np.int64)
        offs_flat = (np.arange(r, dtype=np.int64) * v + lb).astype(np.int32)
        offs = np.ascontiguousarray(offs_flat.reshape(rb_n, P).T)
        in_maps.append(
            {
                "x": xs,
                "offs": offs,
                "iota1": io1.reshape(P, 1),
                "iota2": io2.reshape(P, 1),
                "iota3": io3.reshape(P, 1),
            }
        )
    return in_maps


def run(inputs, trace=False, f=4000):
    from concourse.bass_utils import run_bass_kernel_spmd

    x_full = np.asarray(inputs["outputs"], dtype=np.float32)
    labels_full = np.asarray(inputs["labels"])
    n, v = x_full.shape
    r = n // N_CORES
    nc = build_nc(N_CORES, r, v, f)
    in_maps = make_host_inputs(x_full, labels_full, N_CORES, r, v)
    res = run_bass_kernel_spmd(
        nc, in_maps, list(range(N_CORES)), trace=trace
    )
    val = np.asarray(res.results[0]["out"], dtype=np.float32).reshape(-1)[0]
    return np.asarray(val, dtype=np.float32), res


def kernel(outputs=None, labels=None, **_ignored):
    out, _ = run({"outputs": outputs, "labels": labels})
    return out


# revision 23
# speedup vs baseline: 1.3665x; 1.0181x over previous
"""Bottom-k cross-entropy loss on 8 Trainium2 NeuronCores.

Per-sample CE over [8192, 32000] logits, then mean of the 4096 smallest
losses.  Data-parallel: rows sharded across 8 cores; each core streams its
131MB shard once (memory-bound), computes local CE via one fused
exp+accumulate pass on the scalar engine, all-gathers the 8192 losses
(tiny), and every core redundantly runs an exact threshold-refinement
selection (3 rounds x 128 brackets, then a tie-corrected min-sum) to
produce the bottom-k mean.

Selection math: brackets are multiples of 2^-16 < 32, so all threshold
arithmetic is exact in f32.  With t >= v_(m) within one final bracket,
  mean_bottom_m = (sum_i min(x_i, t) - (N - m) * t) / m
is exact up to (C(t)-m)*bracket_width/m < 1e-7.
"""

import numpy as np

N_CORES = 8
N_FULL, V_FULL = 8192, 32000
P = 128

# bracket steps: ranges 32, 0.25, 2^-9; all CE values lie in (0, 32]
S1, S2, S3 = 2.0**-2, 2.0**-9, 2.0**-16


def build_nc(n_cores, r, v, f):
    """Build the SPMD Bass program (identical on every core)."""
    from concourse import bass, bacc, mybir, tile

    assert r % P == 0 and v % f == 0
    rb_n = r // P
    nch = v // f
    ng = r * n_cores
    m = ng // 2
    f32 = mybir.dt.float32

    nc = bacc.Bacc()
    x = nc.declare_dram_parameter("x", [r, v], f32, isOutput=False)
    offs = nc.declare_dram_parameter("offs", [P, rb_n], mybir.dt.int32, isOutput=False)
    iota1 = nc.declare_dram_parameter("iota1", [P, 1], f32, isOutput=False)
    iota1n = nc.declare_dram_parameter("iota1n", [P, 1], f32, isOutput=False)
    iota2 = nc.declare_dram_parameter("iota2", [P, 1], f32, isOutput=False)
    out = nc.declare_dram_parameter("out", [1, 1], f32, isOutput=True)

    with tile.TileContext(nc) as tc:
        with (
            tc.tile_pool(name="dram", bufs=1, space="DRAM") as dpool,
            tc.tile_pool(name="consts", bufs=1) as cpool,
            tc.tile_pool(name="xs", bufs=5) as xpool,
            tc.tile_pool(name="es", bufs=2) as epool,
            tc.tile_pool(name="stats", bufs=2) as spool,
            tc.tile_pool(name="sel", bufs=1) as selpool,
            tc.tile_pool(name="psum", bufs=2, space="PSUM") as ppool,
        ):
            ce_local = dpool.tile([r, 1], f32, name="ce_local")
            ce_all = dpool.tile([ng, 1], f32, addr_space="Shared", name="ce_all")
            offs_sb = cpool.tile([P, rb_n], mybir.dt.int32)
            nc.gpsimd.dma_start(offs_sb[:], offs[:])
            io1 = cpool.tile([P, 1], f32)
            nc.gpsimd.dma_start(io1[:], iota1[:])
            io1n = cpool.tile([P, 1], f32)
            nc.gpsimd.dma_start(io1n[:], iota1n[:])
            io2 = cpool.tile([P, 1], f32)
            nc.gpsimd.dma_start(io2[:], iota2[:])

            # tiny dummy partition_broadcast: forces the gpsimd ucode library
            # load to happen here (gpsimd is idle during streaming) instead of
            # right before the real broadcast in the latency-critical tail
            dsrc = cpool.tile([1, 4], f32)
            nc.vector.memset(dsrc[:], 0.0)
            dout = cpool.tile([P, 4], f32)
            nc.gpsimd.partition_broadcast(dout[:], dsrc[:])

            # gather picked logits: x.flat[row*v + label] for each local row
            picked = cpool.tile([P, rb_n], f32)
            x_flat = x[:].rearrange("a b -> (a b) ()")
            for rbi in range(rb_n):
                nc.gpsimd.indirect_dma_start(
                    out=picked[:, rbi : rbi + 1],
                    out_offset=None,
                    in_=x_flat,
                    in_offset=bass.IndirectOffsetOnAxis(
                        ap=offs_sb[:, rbi : rbi + 1], axis=0
                    ),
                )

            # streaming pass: pure DMA + fused exp/accumulate, no mid-stream
            # epilogues (table switches and dependent stores would stall the
            # sync HWDGE ring and the ACT queue at row-block boundaries)
            part_all = spool.tile([P, rb_n * nch], f32)
            for rbi in range(rb_n):
                for c in range(nch):
                    xt = xpool.tile([P, f], f32, tag="xt")
                    nc.sync.dma_start(
                        xt[:], x[rbi * P : (rbi + 1) * P, c * f : (c + 1) * f]
                    )
                    esc = epool.tile([P, f], f32, tag="esc")
                    nc.scalar.activation(
                        out=esc[:],
                        in_=xt[:],
                        func=mybir.ActivationFunctionType.Exp,
                        accum_out=part_all[:, rbi * nch + c : rbi * nch + c + 1],
                    )
            # batched epilogue for all row blocks at once
            s_all = spool.tile([P, rb_n], f32)
            nc.vector.tensor_reduce(
                s_all[:],
                part_all[:].rearrange("p (b c) -> p b c", c=nch),
                axis=mybir.AxisListType.X,
                op=mybir.AluOpType.add,
            )
            logz_all = spool.tile([P, rb_n], f32)
            nc.scalar.activation(
                out=logz_all[:], in_=s_all[:], func=mybir.ActivationFunctionType.Ln
            )
            ce_sb = spool.tile([P, rb_n], f32)
            nc.vector.tensor_tensor(
                out=ce_sb[:],
                in0=logz_all[:],
                in1=picked[:],
                op=mybir.AluOpType.subtract,
            )
            # p-major layout into ce_local (any permutation is fine: the
            # bottom-k mean is permutation invariant)
            nc.sync.dma_start(
                ce_local[:].rearrange("(p b) 1 -> p b", b=rb_n), ce_sb[:]
            )

            # all-gather the per-sample losses (tiny)
            nc.gpsimd.collective_compute(
                "AllGather",
                mybir.AluOpType.bypass,
                replica_groups=[list(range(n_cores))],
                ins=[ce_local[:].opt()],
                outs=[ce_all[:].opt()],
            )

            # replicate all ng losses into every partition (gpsimd ucode
            # cross-lane broadcast of partition 0)
            ce_row = selpool.tile([1, ng], f32)
            nc.sync.dma_start(ce_row[:], ce_all[:].rearrange("a 1 -> 1 a"))
            xrep = selpool.tile([P, ng], f32)
            nc.gpsimd.partition_broadcast(xrep[:], ce_row[:])

            # selection: split each whole-array pass between DVE (first half,
            # is_le / min) and ACT (second half, Sign / Relu with per-partition
            # bias) so the two engines run concurrently
            assert ng % 2 == 0 and ng == 2 * m
            h = ng // 2
            xa = xrep[:, :h]
            xb = xrep[:, h:]
            dummy = selpool.tile([P, 1], f32)
            ones = selpool.tile([P, P], f32)
            nc.vector.memset(ones[:], 1.0)

            def count_round(t_ap, tn_ap, name):
                # DVE: cA = #{x_A <= T}
                ca = selpool.tile([P, 1], f32, name=f"ca{name}")
                nc.vector.tensor_scalar(
                    out=dummy[:].broadcast_to([P, h]),
                    in0=xa,
                    scalar1=t_ap,
                    scalar2=None,
                    op0=mybir.AluOpType.is_le,
                    op1=mybir.AluOpType.add,
                    accum_out=ca[:],
                )
                # ACT: sgB = sum sign(x_B - T)  =>  cB = (h - sgB)/2 (ties ~0)
                scr = epool.tile([P, h], f32, tag="esc", name=f"scr{name}")
                sgb = selpool.tile([P, 1], f32, name=f"sgb{name}")
                nc.scalar.activation(
                    out=scr[:],
                    in_=xb,
                    func=mybir.ActivationFunctionType.Sign,
                    bias=tn_ap,
                    scale=1.0,
                    accum_out=sgb[:],
                )
                # u = cA - sgB/2 ; cnt >= m  <=>  u >= m - h/2
                u = selpool.tile([P, 1], f32, name=f"u{name}")
                nc.vector.tensor_scalar(
                    out=u[:], in0=sgb[:], scalar1=-0.5, scalar2=ca[:],
                    op0=mybir.AluOpType.mult, op1=mybir.AluOpType.add,
                )
                ge = selpool.tile([P, 1], f32, name=f"ge{name}")
                nc.vector.tensor_scalar(
                    out=ge[:], in0=u[:], scalar1=float(m) - h / 2.0, scalar2=None,
                    op0=mybir.AluOpType.is_ge,
                )
                # partition-sum of ge, replicated, via ones-matmul
                g = ppool.tile([P, 1], f32, name=f"g{name}", tag="gps")
                nc.tensor.matmul(out=g[:], lhsT=ones[:], rhs=ge[:], start=True, stop=True)
                return g

            # round 1: thresholds io1 = (p+1)*S1; biased lo1 = -g1*S1
            g1 = count_round(io1[:], io1n[:], "1")
            lo1 = selpool.tile([P, 1], f32)
            nc.vector.tensor_scalar(
                out=lo1[:], in0=g1[:], scalar1=-S1, scalar2=None,
                op0=mybir.AluOpType.mult,
            )
            # round 2: T2 = lo1 + (range1 + (p+1)*S2); negT2 = g1*S1 - io2
            t2 = selpool.tile([P, 1], f32)
            nc.vector.tensor_tensor(
                out=t2[:], in0=lo1[:], in1=io2[:], op=mybir.AluOpType.add
            )
            t2n = selpool.tile([P, 1], f32)
            nc.vector.tensor_scalar(
                out=t2n[:], in0=g1[:], scalar1=S1, scalar2=io2[:],
                op0=mybir.AluOpType.mult, op1=mybir.AluOpType.subtract,
            )
            g2 = count_round(t2[:], t2n[:], "2")
            lo2 = selpool.tile([P, 1], f32)
            nc.vector.tensor_scalar(
                out=lo2[:], in0=g2[:], scalar1=-S2, scalar2=lo1[:],
                op0=mybir.AluOpType.mult, op1=mybir.AluOpType.add,
            )
            # final threshold t = true_lo2 + S2 (un-bias by the two ranges).
            # Bracket width S2 ~ 2e-3: the min-sum correction keeps the
            # result error ~ (#ties within S2 of v_m) * S2 / m ~ 1e-6 abs.
            c_t = 128.0 * S1 + 128.0 * S2 + S2
            tf = selpool.tile([P, 1], f32)
            nc.vector.tensor_scalar(
                out=tf[:], in0=lo2[:], scalar1=c_t, scalar2=None,
                op0=mybir.AluOpType.add,
            )
            # bottom-m mean, tie-corrected.  With h = m the t terms cancel:
            #   res = (sum_A min(x,t) - sum_B relu(t-x)) / m
            n_sc = 4
            sc = h // n_sc
            smin_cols = selpool.tile([P, n_sc], f32)
            for k in range(n_sc):
                nc.vector.tensor_scalar(
                    out=dummy[:].broadcast_to([P, sc]),
                    in0=xa[:, k * sc : (k + 1) * sc],
                    scalar1=tf[:],
                    scalar2=None,
                    op0=mybir.AluOpType.min,
                    op1=mybir.AluOpType.add,
                    accum_out=smin_cols[:, k : k + 1],
                )
            relu_cols = selpool.tile([P, n_sc], f32)
            for k in range(n_sc):
                scr = epool.tile([P, sc], f32, tag="esc", name=f"scrr{k}")
                nc.scalar.activation(
                    out=scr[:],
                    in_=xb[:, k * sc : (k + 1) * sc],
                    func=mybir.ActivationFunctionType.Relu,
                    bias=tf[:],
                    scale=-1.0,
                    accum_out=relu_cols[:, k : k + 1],
                )
            smin = selpool.tile([P, 1], f32)
            nc.vector.reduce_sum(smin[:], smin_cols[:], axis=mybir.AxisListType.X)
            srelu = selpool.tile([P, 1], f32)
            nc.vector.reduce_sum(srelu[:], relu_cols[:], axis=mybir.AxisListType.X)
            d = selpool.tile([P, 1], f32)
            nc.vector.tensor_tensor(
                out=d[:], in0=smin[:], in1=srelu[:], op=mybir.AluOpType.subtract
            )
            res = selpool.tile([P, 1], f32)
            nc.vector.tensor_scalar(
                out=res[:], in0=d[:], scalar1=1.0 / m, scalar2=None,
                op0=mybir.AluOpType.mult,
            )
            nc.sync.dma_start(out[:], res[0:1, :])

    if not nc.is_finalized():
        nc.finalize()
    return nc


def make_host_inputs(x_full, labels_full, n_cores, r, v):
    """Shard rows across cores and build the per-core input maps."""
    rb_n = r // P
    io1 = ((np.arange(P, dtype=np.float64) + 1) * S1).astype(np.float32)
    io2 = (128 * S1 + (np.arange(P, dtype=np.float64) + 1) * S2).astype(np.float32)
    in_maps = []
    for c in range(n_cores):
        rows = slice(c * r, (c + 1) * r)
        xs = np.ascontiguousarray(x_full[rows], dtype=np.float32)
        lb = np.asarray(labels_full[rows], dtype=np.int64)
        offs_flat = (np.arange(r, dtype=np.int64) * v + lb).astype(np.int32)
        offs = np.ascontiguousarray(offs_flat.reshape(rb_n, P).T)
        in_maps.append(
            {
                "x": xs,
                "offs": offs,
                "iota1": io1.reshape(P, 1),
                "iota1n": (-io1).reshape(P, 1),
                "iota2": io2.reshape(P, 1),
            }
        )
    return in_maps


def run(inputs, trace=False, f=4000):
    from concourse.bass_utils import run_bass_kernel_spmd

    x_full = np.asarray(inputs["outputs"], dtype=np.float32)
    labels_full = np.asarray(inputs["labels"])
    n, v = x_full.shape
    r = n // N_CORES
    nc = build_nc(N_CORES, r, v, f)
    in_maps = make_host_inputs(x_full, labels_full, N_CORES, r, v)
    res = run_bass_kernel_spmd(
        nc, in_maps, list(range(N_CORES)), trace=trace
    )
    val = np.asarray(res.results[0]["out"], dtype=np.float32).reshape(-1)[0]
    return np.asarray(val, dtype=np.float32), res


def kernel(outputs=None, labels=None, **_ignored):
    out, _ = run({"outputs": outputs, "labels": labels})
    return out


# revision 27
# speedup vs baseline: 1.3944x; 1.0204x over previous
"""Bottom-k cross-entropy loss on 8 Trainium2 NeuronCores.

Per-sample CE over [8192, 32000] logits, then mean of the 4096 smallest
losses.  Data-parallel: rows sharded across 8 cores; each core streams its
131MB shard once (memory-bound), computes local CE via one fused
exp+accumulate pass on the scalar engine, all-gathers the 8192 losses
(tiny), and every core redundantly runs an exact threshold-refinement
selection (3 rounds x 128 brackets, then a tie-corrected min-sum) to
produce the bottom-k mean.

Selection math: brackets are multiples of 2^-16 < 32, so all threshold
arithmetic is exact in f32.  With t >= v_(m) within one final bracket,
  mean_bottom_m = (sum_i min(x_i, t) - (N - m) * t) / m
is exact up to (C(t)-m)*bracket_width/m < 1e-7.
"""

import numpy as np

N_CORES = 8
N_FULL, V_FULL = 8192, 32000
P = 128

# bracket steps: ranges 32, 0.25, 2^-9; all CE values lie in (0, 32]
S1, S2, S3 = 2.0**-2, 2.0**-9, 2.0**-16


def build_nc(n_cores, r, v, f):
    """Build the SPMD Bass program (identical on every core)."""
    from concourse import bass, bacc, mybir, tile

    assert r % P == 0 and v % f == 0
    rb_n = r // P
    nch = v // f
    ng = r * n_cores
    m = ng // 2
    f32 = mybir.dt.float32

    nc = bacc.Bacc()
    x = nc.declare_dram_parameter("x", [r, v], f32, isOutput=False)
    offs = nc.declare_dram_parameter("offs", [P, rb_n], mybir.dt.int32, isOutput=False)
    iota1 = nc.declare_dram_parameter("iota1", [P, 1], f32, isOutput=False)
    iota1n = nc.declare_dram_parameter("iota1n", [P, 1], f32, isOutput=False)
    iota2 = nc.declare_dram_parameter("iota2", [P, 1], f32, isOutput=False)
    out = nc.declare_dram_parameter("out", [1, 1], f32, isOutput=True)

    with tile.TileContext(nc) as tc:
        with (
            tc.tile_pool(name="dram", bufs=1, space="DRAM") as dpool,
            tc.tile_pool(name="consts", bufs=1) as cpool,
            tc.tile_pool(name="xs", bufs=5) as xpool,
            tc.tile_pool(name="es", bufs=2) as epool,
            tc.tile_pool(name="stats", bufs=2) as spool,
            tc.tile_pool(name="sel", bufs=1) as selpool,
            tc.tile_pool(name="psum", bufs=2, space="PSUM") as ppool,
        ):
            ce_local = dpool.tile([r, 1], f32, name="ce_local")
            ce_all = dpool.tile([ng, 1], f32, addr_space="Shared", name="ce_all")
            offs_sb = cpool.tile([P, rb_n], mybir.dt.int32)
            nc.gpsimd.dma_start(offs_sb[:], offs[:])
            io1 = cpool.tile([P, 1], f32)
            nc.gpsimd.dma_start(io1[:], iota1[:])
            io1n = cpool.tile([P, 1], f32)
            nc.gpsimd.dma_start(io1n[:], iota1n[:])
            io2 = cpool.tile([P, 1], f32)
            nc.gpsimd.dma_start(io2[:], iota2[:])

            # tiny dummy partition_broadcast: forces the gpsimd ucode library
            # load to happen here (gpsimd is idle during streaming) instead of
            # right before the real broadcast in the latency-critical tail
            dsrc = cpool.tile([1, 4], f32)
            nc.vector.memset(dsrc[:], 0.0)
            dout = cpool.tile([P, 4], f32)
            nc.gpsimd.partition_broadcast(dout[:], dsrc[:])

            # gather picked logits: x.flat[row*v + label] for each local row
            picked = cpool.tile([P, rb_n], f32)
            x_flat = x[:].rearrange("a b -> (a b) ()")
            for rbi in range(rb_n):
                nc.gpsimd.indirect_dma_start(
                    out=picked[:, rbi : rbi + 1],
                    out_offset=None,
                    in_=x_flat,
                    in_offset=bass.IndirectOffsetOnAxis(
                        ap=offs_sb[:, rbi : rbi + 1], axis=0
                    ),
                )

            # streaming pass: pure DMA + fused exp/accumulate, no mid-stream
            # epilogues (table switches and dependent stores would stall the
            # sync HWDGE ring and the ACT queue at row-block boundaries)
            part_all = spool.tile([P, rb_n * nch], f32)
            for rbi in range(rb_n):
                for c in range(nch):
                    xt = xpool.tile([P, f], f32, tag="xt")
                    nc.sync.dma_start(
                        xt[:], x[rbi * P : (rbi + 1) * P, c * f : (c + 1) * f]
                    )
                    esc = epool.tile([P, f], f32, tag="esc")
                    nc.scalar.activation(
                        out=esc[:],
                        in_=xt[:],
                        func=mybir.ActivationFunctionType.Exp,
                        accum_out=part_all[:, rbi * nch + c : rbi * nch + c + 1],
                    )
            # batched epilogue for all row blocks at once
            s_all = spool.tile([P, rb_n], f32)
            nc.vector.tensor_reduce(
                s_all[:],
                part_all[:].rearrange("p (b c) -> p b c", c=nch),
                axis=mybir.AxisListType.X,
                op=mybir.AluOpType.add,
            )
            logz_all = spool.tile([P, rb_n], f32)
            nc.scalar.activation(
                out=logz_all[:], in_=s_all[:], func=mybir.ActivationFunctionType.Ln
            )
            ce_sb = spool.tile([P, rb_n], f32)
            nc.vector.tensor_tensor(
                out=ce_sb[:],
                in0=logz_all[:],
                in1=picked[:],
                op=mybir.AluOpType.subtract,
            )
            # p-major layout into ce_local (any permutation is fine: the
            # bottom-k mean is permutation invariant)
            nc.sync.dma_start(
                ce_local[:].rearrange("(p b) 1 -> p b", b=rb_n), ce_sb[:]
            )

            # all-gather the per-sample losses (tiny)
            nc.gpsimd.collective_compute(
                "AllGather",
                mybir.AluOpType.bypass,
                replica_groups=[list(range(n_cores))],
                ins=[ce_local[:].opt()],
                outs=[ce_all[:].opt()],
            )

            # replicate all ng losses into every partition (gpsimd ucode
            # cross-lane broadcast of partition 0), in two halves so the
            # DVE can start on the first half while the second broadcasts
            ce_row = selpool.tile([1, ng], f32)
            nc.sync.dma_start(ce_row[:], ce_all[:].rearrange("a 1 -> 1 a"))
            xrep = selpool.tile([P, ng], f32)
            nc.gpsimd.partition_broadcast(xrep[:, : ng // 2], ce_row[:, : ng // 2])
            nc.gpsimd.partition_broadcast(xrep[:, ng // 2 :], ce_row[:, ng // 2 :])

            # selection: split each whole-array pass between DVE (first half,
            # is_le / min) and ACT (second half, Sign / Relu with per-partition
            # bias) so the two engines run concurrently
            assert ng % 2 == 0 and ng == 2 * m
            h = ng // 2
            xa = xrep[:, :h]
            xb = xrep[:, h:]
            dummy = selpool.tile([P, 1], f32)
            ones = selpool.tile([P, P], f32)
            nc.vector.memset(ones[:], 1.0)

            def count_round(t_ap, tn_ap, name):
                # DVE: cA = #{x_A <= T}
                ca = selpool.tile([P, 1], f32, name=f"ca{name}")
                nc.vector.tensor_scalar(
                    out=dummy[:].broadcast_to([P, h]),
                    in0=xa,
                    scalar1=t_ap,
                    scalar2=None,
                    op0=mybir.AluOpType.is_le,
                    op1=mybir.AluOpType.add,
                    accum_out=ca[:],
                )
                # ACT: sgB = sum sign(x_B - T)  =>  cB = (h - sgB)/2 (ties ~0)
                scr = epool.tile([P, h], f32, tag="esc", name=f"scr{name}")
                sgb = selpool.tile([P, 1], f32, name=f"sgb{name}")
                nc.scalar.activation(
                    out=scr[:],
                    in_=xb,
                    func=mybir.ActivationFunctionType.Sign,
                    bias=tn_ap,
                    scale=1.0,
                    accum_out=sgb[:],
                )
                # u = cA - sgB/2 ; cnt >= m  <=>  u >= m - h/2
                u = selpool.tile([P, 1], f32, name=f"u{name}")
                nc.vector.tensor_scalar(
                    out=u[:], in0=sgb[:], scalar1=-0.5, scalar2=ca[:],
                    op0=mybir.AluOpType.mult, op1=mybir.AluOpType.add,
                )
                ge = selpool.tile([P, 1], f32, name=f"ge{name}")
                nc.vector.tensor_scalar(
                    out=ge[:], in0=u[:], scalar1=float(m) - h / 2.0, scalar2=None,
                    op0=mybir.AluOpType.is_ge,
                )
                # partition-sum of ge, replicated, via ones-matmul
                g = ppool.tile([P, 1], f32, name=f"g{name}", tag="gps")
                nc.tensor.matmul(out=g[:], lhsT=ones[:], rhs=ge[:], start=True, stop=True)
                return g

            # round 1: thresholds io1 = (p+1)*S1; biased lo1 = -g1*S1
            g1 = count_round(io1[:], io1n[:], "1")
            lo1 = selpool.tile([P, 1], f32)
            nc.vector.tensor_scalar(
                out=lo1[:], in0=g1[:], scalar1=-S1, scalar2=None,
                op0=mybir.AluOpType.mult,
            )
            # round 2: T2 = lo1 + (range1 + (p+1)*S2); negT2 = g1*S1 - io2
            t2 = selpool.tile([P, 1], f32)
            nc.vector.tensor_tensor(
                out=t2[:], in0=lo1[:], in1=io2[:], op=mybir.AluOpType.add
            )
            t2n = selpool.tile([P, 1], f32)
            nc.vector.tensor_scalar(
                out=t2n[:], in0=g1[:], scalar1=S1, scalar2=io2[:],
                op0=mybir.AluOpType.mult, op1=mybir.AluOpType.subtract,
            )
            g2 = count_round(t2[:], t2n[:], "2")
            lo2 = selpool.tile([P, 1], f32)
            nc.vector.tensor_scalar(
                out=lo2[:], in0=g2[:], scalar1=-S2, scalar2=lo1[:],
                op0=mybir.AluOpType.mult, op1=mybir.AluOpType.add,
            )
            # final threshold t = true_lo2 + S2 (un-bias by the two ranges).
            # Bracket width S2 ~ 2e-3: the min-sum correction keeps the
            # result error ~ (#ties within S2 of v_m) * S2 / m ~ 1e-6 abs.
            c_t = 128.0 * S1 + 128.0 * S2 + S2
            tf = selpool.tile([P, 1], f32)
            nc.vector.tensor_scalar(
                out=tf[:], in0=lo2[:], scalar1=c_t, scalar2=None,
                op0=mybir.AluOpType.add,
            )
            # bottom-m mean, tie-corrected.  With h = m the t terms cancel:
            #   res = (sum_A min(x,t) - sum_B relu(t-x)) / m
            n_sc = 4
            sc = h // n_sc
            smin_cols = selpool.tile([P, n_sc], f32)
            for k in range(n_sc):
                nc.vector.tensor_scalar(
                    out=dummy[:].broadcast_to([P, sc]),
                    in0=xa[:, k * sc : (k + 1) * sc],
                    scalar1=tf[:],
                    scalar2=None,
                    op0=mybir.AluOpType.min,
                    op1=mybir.AluOpType.add,
                    accum_out=smin_cols[:, k : k + 1],
                )
            relu_cols = selpool.tile([P, n_sc], f32)
            for k in range(n_sc):
                scr = epool.tile([P, sc], f32, tag="esc", name=f"scrr{k}")
                nc.scalar.activation(
                    out=scr[:],
                    in_=xb[:, k * sc : (k + 1) * sc],
                    func=mybir.ActivationFunctionType.Relu,
                    bias=tf[:],
                    scale=-1.0,
                    accum_out=relu_cols[:, k : k + 1],
                )
            smin = selpool.tile([P, 1], f32)
            nc.vector.reduce_sum(smin[:], smin_cols[:], axis=mybir.AxisListType.X)
            srelu = selpool.tile([P, 1], f32)
            nc.vector.reduce_sum(srelu[:], relu_cols[:], axis=mybir.AxisListType.X)
            d = selpool.tile([P, 1], f32)
            nc.vector.tensor_tensor(
                out=d[:], in0=smin[:], in1=srelu[:], op=mybir.AluOpType.subtract
            )
            res = selpool.tile([P, 1], f32)
            nc.vector.tensor_scalar(
                out=res[:], in0=d[:], scalar1=1.0 / m, scalar2=None,
                op0=mybir.AluOpType.mult,
            )
            nc.sync.dma_start(out[:], res[0:1, :])

    if not nc.is_finalized():
        nc.finalize()
    return nc


def make_host_inputs(x_full, labels_full, n_cores, r, v):
    """Shard rows across cores and build the per-core input maps."""
    rb_n = r // P
    io1 = ((np.arange(P, dtype=np.float64) + 1) * S1).astype(np.float32)
    io2 = (128 * S1 + (np.arange(P, dtype=np.float64) + 1) * S2).astype(np.float32)
    in_maps = []
    for c in range(n_cores):
        rows = slice(c * r, (c + 1) * r)
        xs = np.ascontiguousarray(x_full[rows], dtype=np.float32)
        lb = np.asarray(labels_full[rows], dtype=np.int64)
        offs_flat = (np.arange(r, dtype=np.int64) * v + lb).astype(np.int32)
        offs = np.ascontiguousarray(offs_flat.reshape(rb_n, P).T)
        in_maps.append(
            {
                "x": xs,
                "offs": offs,
                "iota1": io1.reshape(P, 1),
                "iota1n": (-io1).reshape(P, 1),
                "iota2": io2.reshape(P, 1),
            }
        )
    return in_maps


def run(inputs, trace=False, f=4000):
    from concourse.bass_utils import run_bass_kernel_spmd

    x_full = np.asarray(inputs["outputs"], dtype=np.float32)
    labels_full = np.asarray(inputs["labels"])
    n, v = x_full.shape
    r = n // N_CORES
    nc = build_nc(N_CORES, r, v, f)
    in_maps = make_host_inputs(x_full, labels_full, N_CORES, r, v)
    try:
        res = run_bass_kernel_spmd(
            nc, in_maps, list(range(N_CORES)), trace=trace
        )
    except Exception:
        # transient device errors (e.g. a wedged core from a prior run)
        # usually clear on retry
        res = run_bass_kernel_spmd(
            nc, in_maps, list(range(N_CORES)), trace=trace
        )
    val = np.asarray(res.results[0]["out"], dtype=np.float32).reshape(-1)[0]
    return np.asarray(val, dtype=np.float32), res


def kernel(outputs=None, labels=None, **_ignored):
    out, _ = run({"outputs": outputs, "labels": labels})
    return out
